# revision 13
# baseline (speedup 1.0000x reference)
"""BWGNN (Bernstein-basis spectral GNN) on 8 Trainium2 NeuronCores.

Math (equivalent to the reference):
    h  = relu(relu(X W1 + b1) W2 + b2)
    L f = f - D^-1/2 A D^-1/2 f        (A via segment-sum of src->dst edges)
    g1 = L h ; g2 = L g1
    out = relu([h|g1|g2] @ W3' + b3) @ W4 + b4
W3' folds the constant Bernstein theta coefficients into W3 (a
compile-time linear reparam of the concat-of-filters formulation).

Distribution: nodes sharded 8 ways (12500/core, padded to 12544 = 98
buckets x 128, degree-sorted within core). Per Laplacian hop:
  - tables of f*dinv (bf16 rows duplicated to 256B) are AllGathered so
    every core holds all rows,
  - each core gathers its in-edge messages with dma_gather (int16 idx,
    4 quartile windows of 25088 rows, 4 SWDGE queues),
  - segment-sum per 128-dst-node bucket: matmul with the gathered
    messages as the stationary operand and a HOST-PRECOMPUTED fp8
    one-hot (dst-position routing) streamed from DRAM as the moving
    operand, accumulating feature-major aggregates in PSUM.
All dense layers and epilogues run feature-major ([64 feat partitions,
node columns]) so pointwise work batches into a few large vector ops
per 7-bucket group; PSUM evacuation rides the idle Scalar engine.
"""
import os
import sys
import types

import numpy as np
import ml_dtypes

import concourse.bacc as bacc
import concourse.bass as bass
import concourse.mybir as mybir
import concourse.tile as tile
from concourse.bass_utils import run_bass_kernel_spmd


# --- antenv.axon_hooks shim (the agent image lacks it; needed only when
# NTFF tracing is requested) ---------------------------------------------
def _install_axon_shim():
    if "antenv.axon_hooks" in sys.modules:
        return
    state = {"hook": None}
    mod = types.ModuleType("antenv.axon_hooks")
    mod.set_axon_ntff_profile_hook = lambda h: state.__setitem__("hook", h)
    mod.get_axon_ntff_profile_hook = lambda: state["hook"]
    sys.modules["antenv.axon_hooks"] = mod
    try:
        import antenv

        antenv.axon_hooks = mod
    except Exception:
        pass
    try:
        from trn_agent_boot.trn_boot import _ntff_profile_via_ctypes

        h = _ntff_profile_via_ctypes("/opt/axon/libaxon_pjrt.so")
        if h is not None:
            mod.set_axon_ntff_profile_hook(h)
    except Exception:
        pass


_install_axon_shim()


# --- BIR fixup: this walrus build rejects >1 sync wait per instruction;
# move excess waits onto preceding InstNoOp carriers (same engine, so
# semantics are unchanged) ------------------------------------------------
def split_waits(nc, max_waits=1):
    for f in nc.m.functions:
        for blk in f.blocks:
            new_insts = []
            for inst in blk.instructions:
                si = inst.sync_info
                if si is not None and len(si.on_wait) > max_waits:
                    waits = list(si.on_wait)
                    extra, keep = waits[:-max_waits], waits[-max_waits:]
                    for i in range(0, len(extra), max_waits):
                        nop = mybir.InstNoOp(
                            name=nc.get_next_instruction_name(), ins=[], outs=[]
                        )
                        nop.engine = inst.engine
                        nop.sync_info = mybir.SyncInfo(
                            on_wait=extra[i : i + max_waits], on_update=[]
                        )
                        nc.register_instruction(nop)
                        new_insts.append(nop)
                    si.on_wait = keep
                new_insts.append(inst)
            blk.instructions[:] = new_insts

N = 100_000
E = 3_200_000
F = 64
C = 2
THETAS = np.array([[3.0, -3.0, 0.75], [0.0, 3.0, -1.5], [0.0, 0.0, 0.75]])
W = 8
RPC = 12500
R = 12544                  # 98 buckets x 128
NB = 98
QW = 25088                 # table rows per quartile window
HALF = 6272                # rows per core per AllGather chunk (49 buckets)
GROUP = 7                  # buckets per gather group
NG = NB // GROUP           # 14 groups
NQUEUE = 4
F2 = 2 * F                 # duplicated bf16 row = 256B

DT_BF16 = mybir.dt.bfloat16
DT_F32 = mybir.dt.float32
DT_I16 = mybir.dt.int16
DT_F8 = mybir.dt.float8e4

LAST_EXEC_NS = None
_TRACE = os.environ.get("BWGNN_TRACE", "0") == "1"


def _preprocess(src, dst):
    src = np.asarray(src).astype(np.int64).ravel()
    dst = np.asarray(dst).astype(np.int64).ravel()
    deg = np.bincount(dst, minlength=N)
    dinv = (np.clip(deg, 1, None).astype(np.float64) ** -0.5).astype(np.float32)

    pos = np.empty(N, dtype=np.int64)
    perm = np.full((W, R), -1, dtype=np.int64)
    for c in range(W):
        g0 = c * RPC
        order = np.argsort(-deg[g0 : g0 + RPC], kind="stable")
        perm[c, :RPC] = g0 + order
        pos[g0 + order] = np.arange(RPC)

    e_c = dst // RPC
    e_pos = pos[dst]
    e_b = e_pos // 128
    e_p = (e_pos % 128).astype(np.int64)
    e_g = e_b // GROUP
    # chunked-AllGather table layout: chunk k = srcpos//HALF gathered
    # separately; window q = k*2 + (srccore//4); offset = (c%4)*HALF + pos%HALF
    s_c = src // RPC
    s_pos = pos[src]
    s_k = s_pos // HALF
    e_q = s_k * 2 + s_c // 4
    e_off = (s_c % 4) * HALF + s_pos % HALF

    key_cbq = (e_c * NB + e_b) * 4 + e_q
    cnt = np.bincount(key_cbq, minlength=W * NB * 4).reshape(W, NB, 4)
    tcnt_bq = np.maximum(1, np.ceil(cnt.max(axis=0) / 128.0).astype(np.int64))

    colof = np.zeros((NG, 4), dtype=np.int64)
    tileof = np.zeros((NG, 4), dtype=np.int64)
    nidx = np.zeros((NG, 4), dtype=np.int64)
    col = til = 0
    for g in range(NG):
        for q in range(4):
            t = int(tcnt_bq[g * GROUP : (g + 1) * GROUP, q].sum())
            colof[g, q], tileof[g, q], nidx[g, q] = col, til, t * 128
            col += t * 8
            til += t
    COLS, NTILES = col, til

    sort_key = ((e_c * NG + e_g) * 4 + e_q) * NB + e_b
    eorder = np.argsort(sort_key, kind="stable")

    idx16 = np.zeros((W, 128, COLS), dtype=np.int16)
    oh8 = np.zeros((W, 128, NTILES * 128), dtype=ml_dtypes.float8_e4m3)
    lanes = np.arange(128, dtype=np.int64)

    for c in range(W):
        eo = eorder[e_c[eorder] == c]
        ptr = 0
        for g in range(NG):
            for q in range(4):
                n = int(nidx[g, q])
                flat_off = np.zeros(n, dtype=np.int64)
                flat_dst = np.full(n, -1, dtype=np.int64)
                fpos = 0
                for b in range(g * GROUP, (g + 1) * GROUP):
                    m = int(cnt[c, b, q])
                    seg = eo[ptr : ptr + m]
                    ptr += m
                    flat_off[fpos : fpos + m] = e_off[seg]
                    flat_dst[fpos : fpos + m] = e_p[seg]
                    fpos += int(tcnt_bq[b, q]) * 128
                wrapped = flat_off.reshape(-1, 16).T.astype(np.int16)
                c0 = int(colof[g, q])
                idx16[c, :, c0 : c0 + n // 16] = np.tile(wrapped, (8, 1))
                # dst one-hot per tile: slot j=(t*128+p) -> (partition p, tile t)
                dl = flat_dst.reshape(-1, 128).T  # [128, ntiles]
                t0 = int(tileof[g, q])
                oh = (dl[:, :, None] == lanes).astype(ml_dtypes.float8_e4m3)
                oh8[c, :, t0 * 128 : (t0 + n // 128) * 128] = oh.reshape(128, -1)
        assert ptr == int((e_c == c).sum())

    return dict(
        dinv=dinv,
        perm=perm,
        tcnt_bq=tcnt_bq,
        colof=colof,
        tileof=tileof,
        nidx=nidx,
        COLS=COLS,
        NTILES=NTILES,
        idx16=idx16,
        oh8=oh8,
    )


def _build(meta):
    tcnt_bq, colof, tileof, nidx = (
        meta["tcnt_bq"],
        meta["colof"],
        meta["tileof"],
        meta["nidx"],
    )
    COLS, NTILES = meta["COLS"], meta["NTILES"]

    nc = bacc.Bacc(None, target_bir_lowering=False, num_swdge_queues=NQUEUE, dynamic_dma_scratch_size=32768)

    xT = nc.declare_dram_parameter("xT", [F, R], DT_BF16, isOutput=False)
    dinvf = nc.declare_dram_parameter("dinvf", [F, R], DT_F32, isOutput=False)
    w1 = nc.declare_dram_parameter("w1", [F, F], DT_BF16, isOutput=False)
    w2 = nc.declare_dram_parameter("w2", [F, F], DT_BF16, isOutput=False)
    w3p = nc.declare_dram_parameter("w3p", [F, 3 * F], DT_BF16, isOutput=False)
    w4 = nc.declare_dram_parameter("w4", [F, C], DT_BF16, isOutput=False)
    b1 = nc.declare_dram_parameter("b1", [F, 1], DT_F32, isOutput=False)
    b2 = nc.declare_dram_parameter("b2", [F, 1], DT_F32, isOutput=False)
    b3 = nc.declare_dram_parameter("b3", [F, 1], DT_F32, isOutput=False)
    b4 = nc.declare_dram_parameter("b4", [C, 1], DT_F32, isOutput=False)
    identb = nc.declare_dram_parameter("identb", [F, F], DT_BF16, isOutput=False)
    idx_in = nc.declare_dram_parameter("idx", [128, COLS], DT_I16, isOutput=False)
    oh_in = nc.declare_dram_parameter("oh8", [128, NTILES * 128], DT_F8, isOutput=False)
    outT = nc.declare_dram_parameter("outT", [NB, C, 128], DT_F32, isOutput=True)

    cc1_in = nc.dram_tensor("cc1_in", [R, F2], DT_BF16)
    t1a = nc.dram_tensor("t1a", [W * HALF, F2], DT_BF16, addr_space="Shared")
    t1b = nc.dram_tensor("t1b", [W * HALF, F2], DT_BF16, addr_space="Shared")
    cc2_in = nc.dram_tensor("cc2_in", [R, F2], DT_BF16)
    t2a = nc.dram_tensor("t2a", [W * HALF, F2], DT_BF16, addr_space="Shared")
    t2b = nc.dram_tensor("t2b", [W * HALF, F2], DT_BF16, addr_space="Shared")
    rg = [list(range(W))]
    RELU = mybir.ActivationFunctionType.Relu
    COPY = mybir.ActivationFunctionType.Copy

    with tile.TileContext(nc) as tc:
        with (
            tc.tile_pool(name="const", bufs=1) as constp,
            tc.tile_pool(name="persist", bufs=1) as persist,
        ):
            def cload(nm, shape, dtype, srcap):
                t = constp.tile(shape, dtype, name=nm, tag=nm)
                nc.sync.dma_start(out=t[:], in_=srcap)
                return t

            w1s = cload("w1s", [F, F], DT_BF16, w1[:])
            w2s = cload("w2s", [F, F], DT_BF16, w2[:])
            w3s = cload("w3s", [F, 3 * F], DT_BF16, w3p[:])
            w4s = cload("w4s", [F, C], DT_BF16, w4[:])
            b1s = cload("b1s", [F, 1], DT_F32, b1[:])
            b2s = cload("b2s", [F, 1], DT_F32, b2[:])
            b3s = cload("b3s", [F, 1], DT_F32, b3[:])
            b4s = cload("b4s", [C, 1], DT_F32, b4[:])
            identbs = cload("identbs", [F, F], DT_BF16, identb[:])

            h_bf = persist.tile([F, R], DT_BF16)
            g1_bf = persist.tile([F, R], DT_BF16)

            # ---------------- phase A: dense layers, feature-major ----------
            with (
                tc.tile_pool(name="pA", bufs=3) as pA,
                tc.tile_pool(name="psA", bufs=2, space="PSUM") as psA,
            ):
                CH = 512  # 4 buckets per chunk
                for k in range(25):
                    c0 = k * CH
                    cw = min(CH, R - c0)
                    sl = slice(c0, c0 + cw)
                    xt = pA.tile([F, CH], DT_BF16, tag="xt")
                    nc.sync.dma_start(out=xt[:, :cw], in_=xT[:, sl])
                    ps1 = psA.tile([F, CH], DT_F32, tag="ps1")
                    nc.tensor.matmul(ps1[:, :cw], w1s[:], xt[:, :cw], start=True, stop=True)
                    h1 = pA.tile([F, CH], DT_BF16, tag="h1")
                    nc.scalar.activation(h1[:, :cw], ps1[:, :cw], RELU, bias=b1s[:])
                    ps2 = psA.tile([F, CH], DT_F32, tag="ps2")
                    nc.tensor.matmul(ps2[:, :cw], w2s[:], h1[:, :cw], start=True, stop=True)
                    nc.scalar.activation(h_bf[:, sl], ps2[:, :cw], RELU, bias=b2s[:])
                    dvc = pA.tile([F, CH], DT_F32, tag="dvc")
                    nc.sync.dma_start(out=dvc[:, :cw], in_=dinvf[:, sl])
                    tw = pA.tile([F, CH], DT_BF16, tag="tw")
                    nc.vector.tensor_tensor(
                        out=tw[:, :cw], in0=h_bf[:, sl], in1=dvc[:, :cw],
                        op=mybir.AluOpType.mult,
                    )
                    for j in range(cw // 128):
                        b = k * 4 + j
                        pst = psA.tile([128, F], DT_BF16, tag="pst")
                        nc.tensor.transpose(
                            pst[:], tw[:, j * 128 : (j + 1) * 128], identbs[:]
                        )
                        twT = pA.tile([128, F2], DT_BF16, tag="twT")
                        nc.scalar.activation(twT[:, 0:F], pst[:], COPY)
                        nc.vector.tensor_copy(out=twT[:, F:F2], in_=pst[:])
                        nc.scalar.dma_start(
                            out=cc1_in[b * 128 : (b + 1) * 128, :], in_=twT[:]
                        )
                    if k == 12:
                        nc.gpsimd.collective_compute(
                            "AllGather",
                            mybir.AluOpType.bypass,
                            replica_groups=rg,
                            ins=[cc1_in[0:HALF, :]],
                            outs=[t1a[:]],
                        )

            nc.gpsimd.collective_compute(
                "AllGather",
                mybir.AluOpType.bypass,
                replica_groups=rg,
                ins=[cc1_in[HALF:R, :]],
                outs=[t1b[:]],
            )

            # tile index -> bucket-in-group map, per (g, q); and K-way part
            # splits (tile-aligned) so each (g,q) gather is issued as K
            # sub-gathers round-robined across the 4 SWDGE queues.
            TPART = 8  # tiles (1024 idx) per sub-gather: small equal parts
            tile_bucket = {}
            parts = {}
            for g in range(NG):
                for q in range(4):
                    ntq = int(nidx[g, q]) // 128
                    tb_map = []
                    for j in range(GROUP):
                        tb_map += [j] * int(tcnt_bq[g * GROUP + j, q])
                    assert len(tb_map) == ntq
                    tile_bucket[(g, q)] = tb_map
                    parts[(g, q)] = [
                        (p0, min(TPART, ntq - p0))
                        for p0 in range(0, ntq, TPART)
                    ]
            KMAX = {
                g: max(len(parts[(g, q)]) for q in range(4)) for g in range(NG)
            }

            def hop(tabA, tabB, epilogue, post_group=None):
                qrr = [0]
                with (
                    tc.tile_pool(name="ixp", bufs=8) as ixp,
                    tc.tile_pool(name="gpl", bufs=12) as gpool,
                    tc.tile_pool(name="ohp", bufs=12) as ohp,
                    tc.tile_pool(name="psB", bufs=2, space="PSUM") as psB,
                    tc.tile_pool(name="epi", bufs=2) as epi,
                    tc.tile_pool(name="psE", bufs=2, space="PSUM") as psE,
                ):
                    for g in range(NG):
                        dvc = epi.tile([F, GROUP * 128], DT_F32, tag="dvc")
                        nc.sync.dma_start(
                            out=dvc[:],
                            in_=dinvf[:, g * GROUP * 128 : (g + 1) * GROUP * 128],
                        )
                        bankA = psB.tile([F, 4 * 128], DT_F32, tag="bka")
                        bankB = psB.tile([F, 3 * 128], DT_F32, tag="bkb")

                        def agg_sl(j):
                            if j < 4:
                                return bankA[:, j * 128 : (j + 1) * 128]
                            return bankB[:, (j - 4) * 128 : (j - 3) * 128]

                        # PE-order emission sequence: (k, q, tile) -> bucket.
                        # Find first/last matmul touching each PSUM bank so
                        # start (whole-bank zero) and stop flags land there.
                        seq = []
                        for k in range(KMAX[g]):
                            for q in range(4):
                                if k >= len(parts[(g, q)]):
                                    continue
                                p0, pt = parts[(g, q)][k]
                                for ti in range(p0, p0 + pt):
                                    seq.append((k, q, ti))
                        first = {}
                        last = {}
                        for i, (k, q, ti) in enumerate(seq):
                            bank = 0 if tile_bucket[(g, q)][ti] < 4 else 1
                            if bank not in first:
                                first[bank] = i
                            last[bank] = i

                        ixs = {}
                        for q in range(4):
                            n = int(nidx[g, q])
                            c0 = int(colof[g, q])
                            ix = ixp.tile([128, n // 16], DT_I16, tag="ix")
                            nc.sync.dma_start(
                                out=ix[:], in_=idx_in[:, c0 : c0 + n // 16]
                            )
                            ixs[q] = ix

                        i = 0
                        for k in range(KMAX[g]):
                            for q in range(4):
                                if k >= len(parts[(g, q)]):
                                    continue
                                p0, pt = parts[(g, q)][k]
                                if pt == 0:
                                    continue
                                npart = pt * 128
                                gt = gpool.tile(
                                    [128, pt * F2], DT_BF16, tag="g"
                                )
                                wt = tabA if q < 2 else tabB
                                w0 = (q % 2) * QW
                                nc.gpsimd.dma_gather(
                                    gt[:].rearrange("p (c f) -> p c f", f=F2),
                                    wt[w0 : w0 + QW, :],
                                    ixs[q][:, p0 * 8 : (p0 + pt) * 8],
                                    num_idxs=npart,
                                    num_idxs_reg=npart,
                                    elem_size=F2,
                                    single_packet=False,
                                    queue_num=qrr[0] % NQUEUE,
                                )
                                qrr[0] += 1
                                t0 = int(tileof[g, q]) + p0
                                oh = ohp.tile([128, pt * 128], DT_F8, tag="oh")
                                nc.sync.dma_start(
                                    out=oh[:],
                                    in_=oh_in[:, t0 * 128 : (t0 + pt) * 128],
                                )
                                for tt in range(pt):
                                    j = tile_bucket[(g, q)][p0 + tt]
                                    bank = 0 if j < 4 else 1
                                    nc.tensor.matmul(
                                        agg_sl(j),
                                        gt[:, tt * F2 : tt * F2 + F],
                                        oh[:, tt * 128 : (tt + 1) * 128],
                                        start=(i == first[bank]),
                                        stop=(i == last[bank]),
                                        skip_group_check=True,
                                    )
                                    i += 1
                        assert i == len(seq)
                        aggs = epi.tile([F, GROUP * 128], DT_F32, tag="aggs")
                        nc.scalar.activation(aggs[:, 0 : 4 * 128], bankA[:], COPY)
                        nc.scalar.activation(
                            aggs[:, 4 * 128 : GROUP * 128], bankB[:], COPY
                        )
                        epilogue(g, aggs, dvc, epi, psE)
                        if post_group is not None:
                            post_group(g)

            # ---------------- phase B: hop 1 ----------------
            def epi_B(g, aggs, dvc, epi, psE):
                gsl = slice(g * GROUP * 128, (g + 1) * GROUP * 128)
                t1 = epi.tile([F, GROUP * 128], DT_F32, tag="t1")
                nc.vector.tensor_tensor(
                    out=t1[:], in0=aggs[:], in1=dvc[:], op=mybir.AluOpType.mult
                )
                nc.vector.tensor_tensor(
                    out=g1_bf[:, gsl], in0=h_bf[:, gsl], in1=t1[:],
                    op=mybir.AluOpType.subtract,
                )
                tw2 = epi.tile([F, GROUP * 128], DT_BF16, tag="tw2")
                nc.vector.tensor_tensor(
                    out=tw2[:], in0=g1_bf[:, gsl], in1=dvc[:],
                    op=mybir.AluOpType.mult,
                )
                for j in range(GROUP):
                    b = g * GROUP + j
                    pst = psE.tile([128, F], DT_BF16, tag="pst")
                    nc.tensor.transpose(
                        pst[:], tw2[:, j * 128 : (j + 1) * 128], identbs[:]
                    )
                    twT = epi.tile([128, F2], DT_BF16, tag="twT")
                    nc.scalar.activation(twT[:, 0:F], pst[:], COPY)
                    nc.vector.tensor_copy(out=twT[:, F:F2], in_=pst[:])
                    nc.scalar.dma_start(
                        out=cc2_in[b * 128 : (b + 1) * 128, :], in_=twT[:]
                    )

            def post_B(g):
                # groups 0-6 cover cc2 rows [0:HALF]; AllGather each half as
                # soon as its producer groups are done so the collective
                # overlaps the rest of hop 1.
                if g == NG // 2 - 1:
                    nc.gpsimd.collective_compute(
                        "AllGather",
                        mybir.AluOpType.bypass,
                        replica_groups=rg,
                        ins=[cc2_in[0:HALF, :]],
                        outs=[t2a[:]],
                    )
                elif g == NG - 1:
                    nc.gpsimd.collective_compute(
                        "AllGather",
                        mybir.AluOpType.bypass,
                        replica_groups=rg,
                        ins=[cc2_in[HALF:R, :]],
                        outs=[t2b[:]],
                    )

            hop(t1a, t1b, epi_B, post_group=post_B)

            # ---------------- phase C: hop 2 + final dense ----------------
            def epi_C(g, aggs, dvc, epi, psE):
                gsl = slice(g * GROUP * 128, (g + 1) * GROUP * 128)
                t2 = epi.tile([F, GROUP * 128], DT_F32, tag="t1")
                nc.vector.tensor_tensor(
                    out=t2[:], in0=aggs[:], in1=dvc[:], op=mybir.AluOpType.mult
                )
                g2 = epi.tile([F, GROUP * 128], DT_BF16, tag="g2")
                nc.vector.tensor_tensor(
                    out=g2[:], in0=g1_bf[:, gsl], in1=t2[:],
                    op=mybir.AluOpType.subtract,
                )
                for j in range(GROUP):
                    b = g * GROUP + j
                    bsl = slice(b * 128, (b + 1) * 128)
                    psZ = psE.tile([F, 128], DT_F32, tag="psZ")
                    nc.tensor.matmul(
                        psZ[:], w3s[:, 0:F], h_bf[:, bsl], start=True, stop=False
                    )
                    nc.tensor.matmul(
                        psZ[:], w3s[:, F : 2 * F], g1_bf[:, bsl],
                        start=False, stop=False,
                    )
                    nc.tensor.matmul(
                        psZ[:], w3s[:, 2 * F : 3 * F],
                        g2[:, j * 128 : (j + 1) * 128],
                        start=False, stop=True,
                    )
                    zb = epi.tile([F, 128], DT_BF16, tag="zb")
                    nc.scalar.activation(zb[:], psZ[:], RELU, bias=b3s[:])
                    psO = psE.tile([C, 128], DT_F32, tag="psO")
                    nc.tensor.matmul(psO[:], w4s[:], zb[:], start=True, stop=True)
                    oadd = epi.tile([C, 128], DT_F32, tag="oadd")
                    nc.vector.tensor_scalar(
                        out=oadd[:],
                        in0=psO[:],
                        scalar1=b4s[:],
                        scalar2=None,
                        op0=mybir.AluOpType.add,
                    )
                    nc.scalar.dma_start(out=outT[b, :, :], in_=oadd[:])

            hop(t2a, t2b, epi_C)

    nc.compile()
    split_waits(nc)
    return nc


def kernel(in_feat, src, dst, W1, b1, W2, b2, W3, b3, W4, b4):
    global LAST_EXEC_NS
    in_feat = np.asarray(in_feat, dtype=np.float32)
    meta = _preprocess(src, dst)
    nc = _build(meta)

    dinv, perm = meta["dinv"], meta["perm"]
    W1 = np.asarray(W1, np.float32)
    W2 = np.asarray(W2, np.float32)
    W3 = np.asarray(W3, np.float32)
    W4 = np.asarray(W4, np.float32)
    b1v = np.asarray(b1, np.float32).reshape(F, 1)
    b2v = np.asarray(b2, np.float32).reshape(F, 1)
    b3v = np.asarray(b3, np.float32).reshape(F, 1)
    b4v = np.asarray(b4, np.float32).reshape(C, 1)
    w3p = np.zeros((F, 3 * F), np.float32)
    for j in range(3):
        acc = np.zeros((F, F), np.float32)
        for i in range(3):
            acc += THETAS[i, j] * W3[i * F : (i + 1) * F, :]
        w3p[:, j * F : (j + 1) * F] = acc

    identb = np.eye(F, dtype=ml_dtypes.bfloat16)

    in_maps = []
    for c in range(W):
        pm = perm[c]
        real = pm >= 0
        xTc = np.zeros((R, F), np.float32)
        xTc[real] = in_feat[pm[real]]
        dv = np.zeros(R, np.float32)
        dv[real] = dinv[pm[real]]
        in_maps.append(
            {
                "xT": np.ascontiguousarray(xTc.T).astype(ml_dtypes.bfloat16),
                "dinvf": np.ascontiguousarray(
                    np.broadcast_to(dv, (F, R))
                ).astype(np.float32),
                "w1": W1.astype(ml_dtypes.bfloat16),
                "w2": W2.astype(ml_dtypes.bfloat16),
                "w3p": w3p.astype(ml_dtypes.bfloat16),
                "w4": W4.astype(ml_dtypes.bfloat16),
                "b1": b1v,
                "b2": b2v,
                "b3": b3v,
                "b4": b4v,
                "identb": identb,
                "idx": meta["idx16"][c],
                "oh8": meta["oh8"][c],
            }
        )

    res = run_bass_kernel_spmd(nc, in_maps, core_ids=list(range(W)), trace=_TRACE)
    LAST_EXEC_NS = res.exec_time_ns

    out = np.empty((N, C), dtype=np.float32)
    for c in range(W):
        oT = res.results[c]["outT"]  # [NB, C, 128]
        om = np.transpose(oT, (0, 2, 1)).reshape(R, C)  # processed order
        pm = perm[c]
        real = pm >= 0
        out[pm[real]] = om[real]
    return out


# revision 14
# speedup vs baseline: 1.0139x; 1.0139x over previous
"""BWGNN (Bernstein-basis spectral GNN) on 8 Trainium2 NeuronCores.

Math (equivalent to the reference):
    h  = relu(relu(X W1 + b1) W2 + b2)
    L f = f - D^-1/2 A D^-1/2 f        (A via segment-sum of src->dst edges)
    g1 = L h ; g2 = L g1
    out = relu([h|g1|g2] @ W3' + b3) @ W4 + b4
W3' folds the constant Bernstein theta coefficients into W3 (a
compile-time linear reparam of the concat-of-filters formulation).

Distribution: nodes sharded 8 ways (12500/core, padded to 12544 = 98
buckets x 128, degree-sorted within core). Per Laplacian hop:
  - tables of f*dinv (bf16 rows duplicated to 256B) are AllGathered so
    every core holds all rows,
  - each core gathers its in-edge messages with dma_gather (int16 idx,
    4 quartile windows of 25088 rows, 4 SWDGE queues),
  - segment-sum per 128-dst-node bucket: matmul with the gathered
    messages as the stationary operand and a HOST-PRECOMPUTED fp8
    one-hot (dst-position routing) streamed from DRAM as the moving
    operand, accumulating feature-major aggregates in PSUM.
All dense layers and epilogues run feature-major ([64 feat partitions,
node columns]) so pointwise work batches into a few large vector ops
per 7-bucket group; PSUM evacuation rides the idle Scalar engine.
"""
import os
import sys
import types

import numpy as np
import ml_dtypes

import concourse.bacc as bacc
import concourse.bass as bass
import concourse.mybir as mybir
import concourse.tile as tile
from concourse.bass_utils import run_bass_kernel_spmd


# --- antenv.axon_hooks shim (the agent image lacks it; needed only when
# NTFF tracing is requested) ---------------------------------------------
def _install_axon_shim():
    if "antenv.axon_hooks" in sys.modules:
        return
    state = {"hook": None}
    mod = types.ModuleType("antenv.axon_hooks")
    mod.set_axon_ntff_profile_hook = lambda h: state.__setitem__("hook", h)
    mod.get_axon_ntff_profile_hook = lambda: state["hook"]
    sys.modules["antenv.axon_hooks"] = mod
    try:
        import antenv

        antenv.axon_hooks = mod
    except Exception:
        pass
    try:
        from trn_agent_boot.trn_boot import _ntff_profile_via_ctypes

        h = _ntff_profile_via_ctypes("/opt/axon/libaxon_pjrt.so")
        if h is not None:
            mod.set_axon_ntff_profile_hook(h)
    except Exception:
        pass


_install_axon_shim()


# --- BIR fixup: this walrus build rejects >1 sync wait per instruction;
# move excess waits onto preceding InstNoOp carriers (same engine, so
# semantics are unchanged) ------------------------------------------------
def split_waits(nc, max_waits=1):
    for f in nc.m.functions:
        for blk in f.blocks:
            new_insts = []
            for inst in blk.instructions:
                si = inst.sync_info
                if si is not None and len(si.on_wait) > max_waits:
                    waits = list(si.on_wait)
                    extra, keep = waits[:-max_waits], waits[-max_waits:]
                    for i in range(0, len(extra), max_waits):
                        nop = mybir.InstNoOp(
                            name=nc.get_next_instruction_name(), ins=[], outs=[]
                        )
                        nop.engine = inst.engine
                        nop.sync_info = mybir.SyncInfo(
                            on_wait=extra[i : i + max_waits], on_update=[]
                        )
                        nc.register_instruction(nop)
                        new_insts.append(nop)
                    si.on_wait = keep
                new_insts.append(inst)
            blk.instructions[:] = new_insts

N = 100_000
E = 3_200_000
F = 64
C = 2
THETAS = np.array([[3.0, -3.0, 0.75], [0.0, 3.0, -1.5], [0.0, 0.0, 0.75]])
W = 8
RPC = 12500
R = 12544                  # 98 buckets x 128
NB = 98
QW = 25088                 # table rows per quartile window
HALF = 6272                # rows per core per AllGather chunk (49 buckets)
GROUP = 7                  # buckets per gather group
NG = NB // GROUP           # 14 groups
NQUEUE = 4
F2 = 2 * F                 # duplicated bf16 row = 256B

DT_BF16 = mybir.dt.bfloat16
DT_F32 = mybir.dt.float32
DT_I16 = mybir.dt.int16
DT_F8 = mybir.dt.float8e4

LAST_EXEC_NS = None
_TRACE = os.environ.get("BWGNN_TRACE", "0") == "1"


def _preprocess(src, dst):
    src = np.asarray(src).astype(np.int64).ravel()
    dst = np.asarray(dst).astype(np.int64).ravel()
    deg = np.bincount(dst, minlength=N)
    dinv = (np.clip(deg, 1, None).astype(np.float64) ** -0.5).astype(np.float32)

    pos = np.empty(N, dtype=np.int64)
    perm = np.full((W, R), -1, dtype=np.int64)
    for c in range(W):
        g0 = c * RPC
        order = np.argsort(-deg[g0 : g0 + RPC], kind="stable")
        perm[c, :RPC] = g0 + order
        pos[g0 + order] = np.arange(RPC)

    e_c = dst // RPC
    e_pos = pos[dst]
    e_b = e_pos // 128
    e_p = (e_pos % 128).astype(np.int64)
    e_g = e_b // GROUP
    # chunked-AllGather table layout: chunk k = srcpos//HALF gathered
    # separately; window q = k*2 + (srccore//4); offset = (c%4)*HALF + pos%HALF
    s_c = src // RPC
    s_pos = pos[src]
    s_k = s_pos // HALF
    e_q = s_k * 2 + s_c // 4
    e_off = (s_c % 4) * HALF + s_pos % HALF

    key_cbq = (e_c * NB + e_b) * 4 + e_q
    cnt = np.bincount(key_cbq, minlength=W * NB * 4).reshape(W, NB, 4)
    tcnt_bq = np.maximum(1, np.ceil(cnt.max(axis=0) / 128.0).astype(np.int64))

    colof = np.zeros((NG, 4), dtype=np.int64)
    tileof = np.zeros((NG, 4), dtype=np.int64)
    nidx = np.zeros((NG, 4), dtype=np.int64)
    col = til = 0
    for g in range(NG):
        for q in range(4):
            t = int(tcnt_bq[g * GROUP : (g + 1) * GROUP, q].sum())
            colof[g, q], tileof[g, q], nidx[g, q] = col, til, t * 128
            col += t * 8
            til += t
    COLS, NTILES = col, til

    sort_key = ((e_c * NG + e_g) * 4 + e_q) * NB + e_b
    eorder = np.argsort(sort_key, kind="stable")

    idx16 = np.zeros((W, 128, COLS), dtype=np.int16)
    oh8 = np.zeros((W, 128, NTILES * 128), dtype=ml_dtypes.float8_e4m3)
    lanes = np.arange(128, dtype=np.int64)

    for c in range(W):
        eo = eorder[e_c[eorder] == c]
        ptr = 0
        for g in range(NG):
            for q in range(4):
                n = int(nidx[g, q])
                flat_off = np.zeros(n, dtype=np.int64)
                flat_dst = np.full(n, -1, dtype=np.int64)
                fpos = 0
                for b in range(g * GROUP, (g + 1) * GROUP):
                    m = int(cnt[c, b, q])
                    seg = eo[ptr : ptr + m]
                    ptr += m
                    flat_off[fpos : fpos + m] = e_off[seg]
                    flat_dst[fpos : fpos + m] = e_p[seg]
                    fpos += int(tcnt_bq[b, q]) * 128
                wrapped = flat_off.reshape(-1, 16).T.astype(np.int16)
                c0 = int(colof[g, q])
                idx16[c, :, c0 : c0 + n // 16] = np.tile(wrapped, (8, 1))
                # dst one-hot per tile: slot j=(t*128+p) -> (partition p, tile t)
                dl = flat_dst.reshape(-1, 128).T  # [128, ntiles]
                t0 = int(tileof[g, q])
                oh = (dl[:, :, None] == lanes).astype(ml_dtypes.float8_e4m3)
                oh8[c, :, t0 * 128 : (t0 + n // 128) * 128] = oh.reshape(128, -1)
        assert ptr == int((e_c == c).sum())

    return dict(
        dinv=dinv,
        perm=perm,
        tcnt_bq=tcnt_bq,
        colof=colof,
        tileof=tileof,
        nidx=nidx,
        COLS=COLS,
        NTILES=NTILES,
        idx16=idx16,
        oh8=oh8,
    )


def _build(meta):
    tcnt_bq, colof, tileof, nidx = (
        meta["tcnt_bq"],
        meta["colof"],
        meta["tileof"],
        meta["nidx"],
    )
    COLS, NTILES = meta["COLS"], meta["NTILES"]

    nc = bacc.Bacc(None, target_bir_lowering=False, num_swdge_queues=NQUEUE, dynamic_dma_scratch_size=32768)

    xT = nc.declare_dram_parameter("xT", [F, R], DT_BF16, isOutput=False)
    dinvf = nc.declare_dram_parameter("dinvf", [F, R], DT_F32, isOutput=False)
    w1 = nc.declare_dram_parameter("w1", [F, F], DT_BF16, isOutput=False)
    w2 = nc.declare_dram_parameter("w2", [F, F], DT_BF16, isOutput=False)
    w3p = nc.declare_dram_parameter("w3p", [F, 3 * F], DT_BF16, isOutput=False)
    w4 = nc.declare_dram_parameter("w4", [F, C], DT_BF16, isOutput=False)
    b1 = nc.declare_dram_parameter("b1", [F, 1], DT_F32, isOutput=False)
    b2 = nc.declare_dram_parameter("b2", [F, 1], DT_F32, isOutput=False)
    b3 = nc.declare_dram_parameter("b3", [F, 1], DT_F32, isOutput=False)
    b4 = nc.declare_dram_parameter("b4", [C, 1], DT_F32, isOutput=False)
    identb = nc.declare_dram_parameter("identb", [F, F], DT_BF16, isOutput=False)
    idx_in = nc.declare_dram_parameter("idx", [128, COLS], DT_I16, isOutput=False)
    oh_in = nc.declare_dram_parameter("oh8", [128, NTILES * 128], DT_F8, isOutput=False)
    outT = nc.declare_dram_parameter("outT", [NB, C, 128], DT_F32, isOutput=True)

    cc1_in = nc.dram_tensor("cc1_in", [R, F2], DT_BF16)
    t1a = nc.dram_tensor("t1a", [W * HALF, F2], DT_BF16, addr_space="Shared")
    t1b = nc.dram_tensor("t1b", [W * HALF, F2], DT_BF16, addr_space="Shared")
    cc2_in = nc.dram_tensor("cc2_in", [R, F2], DT_BF16)
    t2a = nc.dram_tensor("t2a", [W * HALF, F2], DT_BF16, addr_space="Shared")
    t2b = nc.dram_tensor("t2b", [W * HALF, F2], DT_BF16, addr_space="Shared")
    rg = [list(range(W))]
    RELU = mybir.ActivationFunctionType.Relu
    COPY = mybir.ActivationFunctionType.Copy

    with tile.TileContext(nc) as tc:
        with (
            tc.tile_pool(name="const", bufs=1) as constp,
            tc.tile_pool(name="persist", bufs=1) as persist,
        ):
            def cload(nm, shape, dtype, srcap):
                t = constp.tile(shape, dtype, name=nm, tag=nm)
                nc.sync.dma_start(out=t[:], in_=srcap)
                return t

            w1s = cload("w1s", [F, F], DT_BF16, w1[:])
            w2s = cload("w2s", [F, F], DT_BF16, w2[:])
            w3s = cload("w3s", [F, 3 * F], DT_BF16, w3p[:])
            w4s = cload("w4s", [F, C], DT_BF16, w4[:])
            b1s = cload("b1s", [F, 1], DT_F32, b1[:])
            b2s = cload("b2s", [F, 1], DT_F32, b2[:])
            b3s = cload("b3s", [F, 1], DT_F32, b3[:])
            b4s = cload("b4s", [C, 1], DT_F32, b4[:])
            identbs = cload("identbs", [F, F], DT_BF16, identb[:])

            h_bf = persist.tile([F, R], DT_BF16)
            g1_bf = persist.tile([F, R], DT_BF16)

            # ---------------- phase A: dense layers, feature-major ----------
            with (
                tc.tile_pool(name="pA", bufs=4) as pA,
                tc.tile_pool(name="psA", bufs=2, space="PSUM") as psA,
                tc.tile_pool(name="psT", bufs=4, space="PSUM") as psT,
            ):
                CH = 512  # 4 buckets per chunk
                for k in range(25):
                    c0 = k * CH
                    cw = min(CH, R - c0)
                    sl = slice(c0, c0 + cw)
                    xt = pA.tile([F, CH], DT_BF16, tag="xt")
                    nc.sync.dma_start(out=xt[:, :cw], in_=xT[:, sl])
                    ps1 = psA.tile([F, CH], DT_F32, tag="ps1")
                    nc.tensor.matmul(ps1[:, :cw], w1s[:], xt[:, :cw], start=True, stop=True)
                    h1 = pA.tile([F, CH], DT_BF16, tag="h1")
                    nc.scalar.activation(h1[:, :cw], ps1[:, :cw], RELU, bias=b1s[:])
                    ps2 = psA.tile([F, CH], DT_F32, tag="ps2")
                    nc.tensor.matmul(ps2[:, :cw], w2s[:], h1[:, :cw], start=True, stop=True)
                    nc.scalar.activation(h_bf[:, sl], ps2[:, :cw], RELU, bias=b2s[:])
                    dvc = pA.tile([F, CH], DT_F32, tag="dvc")
                    nc.sync.dma_start(out=dvc[:, :cw], in_=dinvf[:, sl])
                    tw = pA.tile([F, CH], DT_BF16, tag="tw")
                    nc.vector.tensor_tensor(
                        out=tw[:, :cw], in0=h_bf[:, sl], in1=dvc[:, :cw],
                        op=mybir.AluOpType.mult,
                    )
                    for j in range(cw // 128):
                        b = k * 4 + j
                        pst = psT.tile([128, F], DT_BF16, tag="pst")
                        nc.tensor.transpose(
                            pst[:], tw[:, j * 128 : (j + 1) * 128], identbs[:]
                        )
                        twT = pA.tile([128, F2], DT_BF16, tag="twT")
                        nc.scalar.activation(twT[:, 0:F], pst[:], COPY)
                        nc.vector.tensor_copy(out=twT[:, F:F2], in_=pst[:])
                        nc.scalar.dma_start(
                            out=cc1_in[b * 128 : (b + 1) * 128, :], in_=twT[:]
                        )
                    if k == 12:
                        nc.gpsimd.collective_compute(
                            "AllGather",
                            mybir.AluOpType.bypass,
                            replica_groups=rg,
                            ins=[cc1_in[0:HALF, :]],
                            outs=[t1a[:]],
                        )

            nc.gpsimd.collective_compute(
                "AllGather",
                mybir.AluOpType.bypass,
                replica_groups=rg,
                ins=[cc1_in[HALF:R, :]],
                outs=[t1b[:]],
            )

            # tile index -> bucket-in-group map, per (g, q); and K-way part
            # splits (tile-aligned) so each (g,q) gather is issued as K
            # sub-gathers round-robined across the 4 SWDGE queues.
            TPART = 8  # tiles (1024 idx) per sub-gather: small equal parts
            tile_bucket = {}
            parts = {}
            for g in range(NG):
                for q in range(4):
                    ntq = int(nidx[g, q]) // 128
                    tb_map = []
                    for j in range(GROUP):
                        tb_map += [j] * int(tcnt_bq[g * GROUP + j, q])
                    assert len(tb_map) == ntq
                    tile_bucket[(g, q)] = tb_map
                    parts[(g, q)] = [
                        (p0, min(TPART, ntq - p0))
                        for p0 in range(0, ntq, TPART)
                    ]
            KMAX = {
                g: max(len(parts[(g, q)]) for q in range(4)) for g in range(NG)
            }

            def hop(tabA, tabB, epilogue, post_group=None, mid0=None):
                qrr = [0]
                with (
                    tc.tile_pool(name="ixp", bufs=8) as ixp,
                    tc.tile_pool(name="gpl", bufs=12) as gpool,
                    tc.tile_pool(name="ohp", bufs=12) as ohp,
                    tc.tile_pool(name="psB", bufs=2, space="PSUM") as psB,
                    tc.tile_pool(name="epi", bufs=2) as epi,
                    tc.tile_pool(name="psE", bufs=2, space="PSUM") as psE,
                ):
                    for g in range(NG):
                        dvc = epi.tile([F, GROUP * 128], DT_F32, tag="dvc")
                        nc.sync.dma_start(
                            out=dvc[:],
                            in_=dinvf[:, g * GROUP * 128 : (g + 1) * GROUP * 128],
                        )
                        bankA = psB.tile([F, 4 * 128], DT_F32, tag="bka")
                        bankB = psB.tile([F, 3 * 128], DT_F32, tag="bkb")

                        def agg_sl(j):
                            if j < 4:
                                return bankA[:, j * 128 : (j + 1) * 128]
                            return bankB[:, (j - 4) * 128 : (j - 3) * 128]

                        # PE-order emission sequence: (k, q, tile) -> bucket.
                        # Find first/last matmul touching each PSUM bank so
                        # start (whole-bank zero) and stop flags land there.
                        seq = []
                        if g == 0 and mid0 is not None:
                            kq_s = (
                                [(k, q) for q in (0, 1)
                                 for k in range(len(parts[(g, q)]))]
                                + [(k, q) for q in (2, 3)
                                   for k in range(len(parts[(g, q)]))]
                            )
                        else:
                            kq_s = [
                                (k, q)
                                for k in range(KMAX[g])
                                for q in range(4)
                                if k < len(parts[(g, q)])
                            ]
                        for k, q in kq_s:
                            p0, pt = parts[(g, q)][k]
                            for ti in range(p0, p0 + pt):
                                seq.append((k, q, ti))
                        first = {}
                        last = {}
                        for i, (k, q, ti) in enumerate(seq):
                            bank = 0 if tile_bucket[(g, q)][ti] < 4 else 1
                            if bank not in first:
                                first[bank] = i
                            last[bank] = i

                        ixs = {}
                        for q in range(4):
                            n = int(nidx[g, q])
                            c0 = int(colof[g, q])
                            ix = ixp.tile([128, n // 16], DT_I16, tag="ix")
                            nc.sync.dma_start(
                                out=ix[:], in_=idx_in[:, c0 : c0 + n // 16]
                            )
                            ixs[q] = ix

                        if g == 0 and mid0 is not None:
                            kq_iter = (
                                [(k, q) for q in (0, 1)
                                 for k in range(len(parts[(g, q)]))]
                                + [(None, None)]
                                + [(k, q) for q in (2, 3)
                                   for k in range(len(parts[(g, q)]))]
                            )
                        else:
                            kq_iter = [
                                (k, q)
                                for k in range(KMAX[g])
                                for q in range(4)
                                if k < len(parts[(g, q)])
                            ]
                        i = 0
                        for k, q in kq_iter:
                                if k is None:
                                    mid0(g)
                                    continue
                                p0, pt = parts[(g, q)][k]
                                if pt == 0:
                                    continue
                                npart = pt * 128
                                gt = gpool.tile(
                                    [128, pt * F2], DT_BF16, tag="g"
                                )
                                wt = tabA if q < 2 else tabB
                                w0 = (q % 2) * QW
                                nc.gpsimd.dma_gather(
                                    gt[:].rearrange("p (c f) -> p c f", f=F2),
                                    wt[w0 : w0 + QW, :],
                                    ixs[q][:, p0 * 8 : (p0 + pt) * 8],
                                    num_idxs=npart,
                                    num_idxs_reg=npart,
                                    elem_size=F2,
                                    single_packet=False,
                                    queue_num=qrr[0] % NQUEUE,
                                )
                                qrr[0] += 1
                                t0 = int(tileof[g, q]) + p0
                                oh = ohp.tile([128, pt * 128], DT_F8, tag="oh")
                                nc.sync.dma_start(
                                    out=oh[:],
                                    in_=oh_in[:, t0 * 128 : (t0 + pt) * 128],
                                )
                                for tt in range(pt):
                                    j = tile_bucket[(g, q)][p0 + tt]
                                    bank = 0 if j < 4 else 1
                                    nc.tensor.matmul(
                                        agg_sl(j),
                                        gt[:, tt * F2 : tt * F2 + F],
                                        oh[:, tt * 128 : (tt + 1) * 128],
                                        start=(i == first[bank]),
                                        stop=(i == last[bank]),
                                        skip_group_check=True,
                                    )
                                    i += 1
                        assert i == len(seq)
                        aggs = epi.tile([F, GROUP * 128], DT_F32, tag="aggs")
                        nc.scalar.activation(aggs[:, 0 : 4 * 128], bankA[:], COPY)
                        nc.scalar.activation(
                            aggs[:, 4 * 128 : GROUP * 128], bankB[:], COPY
                        )
                        epilogue(g, aggs, dvc, epi, psE)
                        if post_group is not None:
                            post_group(g)

            # ---------------- phase B: hop 1 ----------------
            def epi_B(g, aggs, dvc, epi, psE):
                gsl = slice(g * GROUP * 128, (g + 1) * GROUP * 128)
                t1 = epi.tile([F, GROUP * 128], DT_F32, tag="t1")
                nc.vector.tensor_tensor(
                    out=t1[:], in0=aggs[:], in1=dvc[:], op=mybir.AluOpType.mult
                )
                nc.vector.tensor_tensor(
                    out=g1_bf[:, gsl], in0=h_bf[:, gsl], in1=t1[:],
                    op=mybir.AluOpType.subtract,
                )
                tw2 = epi.tile([F, GROUP * 128], DT_BF16, tag="tw2")
                nc.vector.tensor_tensor(
                    out=tw2[:], in0=g1_bf[:, gsl], in1=dvc[:],
                    op=mybir.AluOpType.mult,
                )
                for j in range(GROUP):
                    b = g * GROUP + j
                    pst = psE.tile([128, F], DT_BF16, tag="pst")
                    nc.tensor.transpose(
                        pst[:], tw2[:, j * 128 : (j + 1) * 128], identbs[:]
                    )
                    twT = epi.tile([128, F2], DT_BF16, tag="twT")
                    nc.scalar.activation(twT[:, 0:F], pst[:], COPY)
                    nc.vector.tensor_copy(out=twT[:, F:F2], in_=pst[:])
                    nc.scalar.dma_start(
                        out=cc2_in[b * 128 : (b + 1) * 128, :], in_=twT[:]
                    )

            def post_B(g):
                # groups 0-6 cover cc2 rows [0:HALF]; AllGather each half as
                # soon as its producer groups are done so the collective
                # overlaps the rest of hop 1.
                if g == NG // 2 - 1:
                    nc.gpsimd.collective_compute(
                        "AllGather",
                        mybir.AluOpType.bypass,
                        replica_groups=rg,
                        ins=[cc2_in[0:HALF, :]],
                        outs=[t2a[:]],
                    )

            hop(t1a, t1b, epi_B, post_group=post_B)

            # ---------------- phase C: hop 2 + final dense ----------------
            def epi_C(g, aggs, dvc, epi, psE):
                gsl = slice(g * GROUP * 128, (g + 1) * GROUP * 128)
                t2 = epi.tile([F, GROUP * 128], DT_F32, tag="t1")
                nc.vector.tensor_tensor(
                    out=t2[:], in0=aggs[:], in1=dvc[:], op=mybir.AluOpType.mult
                )
                g2 = epi.tile([F, GROUP * 128], DT_BF16, tag="g2")
                nc.vector.tensor_tensor(
                    out=g2[:], in0=g1_bf[:, gsl], in1=t2[:],
                    op=mybir.AluOpType.subtract,
                )
                for j in range(GROUP):
                    b = g * GROUP + j
                    bsl = slice(b * 128, (b + 1) * 128)
                    psZ = psE.tile([F, 128], DT_F32, tag="psZ")
                    nc.tensor.matmul(
                        psZ[:], w3s[:, 0:F], h_bf[:, bsl], start=True, stop=False
                    )
                    nc.tensor.matmul(
                        psZ[:], w3s[:, F : 2 * F], g1_bf[:, bsl],
                        start=False, stop=False,
                    )
                    nc.tensor.matmul(
                        psZ[:], w3s[:, 2 * F : 3 * F],
                        g2[:, j * 128 : (j + 1) * 128],
                        start=False, stop=True,
                    )
                    zb = epi.tile([F, 128], DT_BF16, tag="zb")
                    nc.scalar.activation(zb[:], psZ[:], RELU, bias=b3s[:])
                    psO = psE.tile([C, 128], DT_F32, tag="psO")
                    nc.tensor.matmul(psO[:], w4s[:], zb[:], start=True, stop=True)
                    oadd = epi.tile([C, 128], DT_F32, tag="oadd")
                    nc.vector.tensor_scalar(
                        out=oadd[:],
                        in0=psO[:],
                        scalar1=b4s[:],
                        scalar2=None,
                        op0=mybir.AluOpType.add,
                    )
                    nc.scalar.dma_start(out=outT[b, :, :], in_=oadd[:])

            def mid_C(g):
                nc.gpsimd.collective_compute(
                    "AllGather",
                    mybir.AluOpType.bypass,
                    replica_groups=rg,
                    ins=[cc2_in[HALF:R, :]],
                    outs=[t2b[:]],
                )

            hop(t2a, t2b, epi_C, mid0=mid_C)

    nc.compile()
    split_waits(nc)
    return nc


def kernel(in_feat, src, dst, W1, b1, W2, b2, W3, b3, W4, b4):
    global LAST_EXEC_NS
    in_feat = np.asarray(in_feat, dtype=np.float32)
    meta = _preprocess(src, dst)
    nc = _build(meta)

    dinv, perm = meta["dinv"], meta["perm"]
    W1 = np.asarray(W1, np.float32)
    W2 = np.asarray(W2, np.float32)
    W3 = np.asarray(W3, np.float32)
    W4 = np.asarray(W4, np.float32)
    b1v = np.asarray(b1, np.float32).reshape(F, 1)
    b2v = np.asarray(b2, np.float32).reshape(F, 1)
    b3v = np.asarray(b3, np.float32).reshape(F, 1)
    b4v = np.asarray(b4, np.float32).reshape(C, 1)
    w3p = np.zeros((F, 3 * F), np.float32)
    for j in range(3):
        acc = np.zeros((F, F), np.float32)
        for i in range(3):
            acc += THETAS[i, j] * W3[i * F : (i + 1) * F, :]
        w3p[:, j * F : (j + 1) * F] = acc

    identb = np.eye(F, dtype=ml_dtypes.bfloat16)

    in_maps = []
    for c in range(W):
        pm = perm[c]
        real = pm >= 0
        xTc = np.zeros((R, F), np.float32)
        xTc[real] = in_feat[pm[real]]
        dv = np.zeros(R, np.float32)
        dv[real] = dinv[pm[real]]
        in_maps.append(
            {
                "xT": np.ascontiguousarray(xTc.T).astype(ml_dtypes.bfloat16),
                "dinvf": np.ascontiguousarray(
                    np.broadcast_to(dv, (F, R))
                ).astype(np.float32),
                "w1": W1.astype(ml_dtypes.bfloat16),
                "w2": W2.astype(ml_dtypes.bfloat16),
                "w3p": w3p.astype(ml_dtypes.bfloat16),
                "w4": W4.astype(ml_dtypes.bfloat16),
                "b1": b1v,
                "b2": b2v,
                "b3": b3v,
                "b4": b4v,
                "identb": identb,
                "idx": meta["idx16"][c],
                "oh8": meta["oh8"][c],
            }
        )

    res = run_bass_kernel_spmd(nc, in_maps, core_ids=list(range(W)), trace=_TRACE)
    LAST_EXEC_NS = res.exec_time_ns

    out = np.empty((N, C), dtype=np.float32)
    for c in range(W):
        oT = res.results[c]["outT"]  # [NB, C, 128]
        om = np.transpose(oT, (0, 2, 1)).reshape(R, C)  # processed order
        pm = perm[c]
        real = pm >= 0
        out[pm[real]] = om[real]
    return out
